# revision 5
# baseline (speedup 1.0000x reference)
"""Trainium2 kernel for nn_Dense_RBS_density: rho <- U rho U^T over a batch
of 8 density matrices in the Hamming-weight-2 basis of 32 qubits (dim=496).

The 15 RBS gates act on disjoint qubit pairs, so they fold into one
orthogonal matrix U (host-built). In a permuted basis U is block-diagonal
with <=4x4 blocks, so each 124-row band only needs its own 124x124
diagonal block of U: per core (one batch element), two passes of 16
independent 124-free fp16 matmuls instead of dense 496^3 work.

Per core (one batch element), with rho' = P rho P^T and block-diagonal
B = P U P^T (host-built, fp16):
  mm1: A^T regions = rho'[kt, mt]^T @ B^T[kt]   (16 independent fp16 matmuls)
  at-copies: ps1 regions -> at_sb fp16, region granular on DVE/ACT/Pool
  mm2: out regions = at[mt, kt]^T @ B^T[kt]     (16 independent fp16 matmuls)
  out-copies: ps2 regions -> out_sb fp16, region granular
  out DMAs: one per band, fp16, split over SP/ACT
"""

import itertools
import math

import numpy as np

N_QUBITS = 32
LIST_GATES = [(2 * i, 2 * i + 1) for i in range(15)]
DIM = 496
PT = 124
NT = 4
N_CORES = 8
ROW = DIM + PT  # 620 packed input row: 496 rho' cols + 124 B^T block cols


def _gate_pairs():
    pairs = list(itertools.combinations(range(N_QUBITS), 2))
    idx = {p: k for k, p in enumerate(pairs)}
    out = []
    for a, b in LIST_GATES:
        rot = []
        for p, k in idx.items():
            if (a in p) and (b not in p):
                other = p[0] if p[1] == a else p[1]
                kp = idx[tuple(sorted((other, b)))]
                rot.append((k, kp))
        out.append(rot)
    return out


_GATE_PAIRS = _gate_pairs()


def _build_perm():
    pairs = list(itertools.combinations(range(N_QUBITS), 2))
    idx = {p: k for k, p in enumerate(pairs)}
    perm = []
    for a in range(15):
        for b in range(a + 1, 15):
            for x in range(2):
                for y in range(2):
                    perm.append(idx[(2 * a + x, 2 * b + y)])
    for q in (30, 31):
        for a in range(15):
            perm.append(idx[tuple(sorted((2 * a, q)))])
            perm.append(idx[tuple(sorted((2 * a + 1, q)))])
    for a in range(15):
        perm.append(idx[(2 * a, 2 * a + 1)])
    perm.append(idx[(30, 31)])
    return np.array(perm)


_PERM = _build_perm()
_INV_PERM = np.argsort(_PERM)


def _build_u(angles: np.ndarray) -> np.ndarray:
    u = np.eye(DIM, dtype=np.float64)
    for g, rot in enumerate(_GATE_PAIRS):
        c = math.cos(float(angles[g]))
        s = math.sin(float(angles[g]))
        k = np.array([r[0] for r in rot])
        kp = np.array([r[1] for r in rot])
        rk, rkp = u[k].copy(), u[kp].copy()
        u[k] = c * rk + s * rkp
        u[kp] = -s * rk + c * rkp
    return u


_NC_CACHE = {}

# schedule knobs (tuned by search.py)
CFG = {
    # at-copy mode per sweep: "half" (DVE regions 0-1 + ACT regions 2-3),
    # "dve" or "act" (whole sweep on one engine). GPSIMD cannot read PSUM
    # on real TRN2, so only DVE/ACT move data out of PSUM.
    "at": ("act", "dve", "dve", "act"),
    # out band-copy engine: dve16 or act16 (single instruction per band)
    "out_cp": ("act16", "dve16", "act16", "dve16"),
    # out band -> DMA engine (sync/scalar/gpsimd)
    "out_dma": ("gpsimd", "sync", "gpsimd", "scalar"),
    # input chunk -> DMA engine, in emission order
    "in_dma": ("sync", "scalar", "gpsimd", "sync"),
    # PE emission order of mm2 groups
    "g_order": (0, 2, 1, 3),
}


def _build_bass():
    import concourse.bass as bass
    import concourse.mybir as mybir
    import concourse.tile as tile
    from concourse.bass import MemorySpace

    f16 = mybir.dt.float16
    f32 = mybir.dt.float32

    nc = bass.Bass("TRN2", target_bir_lowering=False, debug=False)
    inp_d = nc.dram_tensor("inp", [DIM, ROW], f16, kind="ExternalInput").ap()
    use32 = any(k.endswith("64") for k in CFG["out_cp"])
    out32_d = (nc.dram_tensor("out32", [DIM, DIM], f32,
                              kind="ExternalOutput").ap() if use32 else None)
    out16_d = nc.dram_tensor("out16", [DIM, DIM], f16,
                             kind="ExternalOutput").ap()

    with tile.TileContext(nc) as tc:
        with (
            tc.tile_pool(name="consts", bufs=1) as consts,
            tc.tile_pool(name="psum", bufs=1, space=MemorySpace.PSUM) as psum,
        ):
            i64 = mybir.dt.int64
            inp_sb = consts.tile([PT, NT, ROW], f16, tag="inp")
            # slot map: (kt, r) -> (engine, slot index within engine tile)
            MODE_ENGS = {"half": ("vector", "vector", "scalar", "scalar"),
                         "dve": ("vector",) * 4, "act": ("scalar",) * 4}
            at_slot = {}
            eng_nslots = {}
            for kt in range(NT):
                for r in range(NT):
                    eng = MODE_ENGS[CFG["at"][kt]][r]
                    at_slot[(kt, r)] = (eng, eng_nslots.get(eng, 0))
                    eng_nslots[eng] = eng_nslots.get(eng, 0) + 1
            at_tiles = {
                eng: consts.tile([PT, n * PT], f16,
                                 tag=f"at_{eng}", name=f"at_{eng}")
                for eng, n in eng_nslots.items()
            }
            out_bands = [
                consts.tile([PT, DIM],
                            f32 if CFG["out_cp"][mt].endswith("64") else f16,
                            tag=f"ob{mt}", name=f"ob{mt}")
                for mt in range(NT)
            ]
            ob_last_a = consts.tile([PT, 3 * PT], f16, tag="ob_la",
                                    name="ob_la")
            ob_last_b = consts.tile([PT, PT], f16, tag="ob_lb",
                                    name="ob_lb")
            warm_sb = consts.tile([PT, 8], f32, tag="warm")
            scratch_sb = consts.tile([PT, 8], f32, tag="scratch")

            def at_region(kt, r):
                eng, slot = at_slot[(kt, r)]
                return at_tiles[eng][:, slot * PT:(slot + 1) * PT]

            dma_is, mm_is = [], []
            cp_eng = []  # (engine_name, instruction) for drain NOPs

            # --- head ---
            warm_i = nc.vector.memset(warm_sb, 0.0)
            cp_eng.append(("vector", warm_i))

            for kt in range(NT):
                eng = CFG["in_dma"][kt]
                dma_is.append(getattr(nc, eng).dma_start(
                    inp_sb[:, kt, :], inp_d[PT * kt:PT * (kt + 1), :]))

            # ACT activation-table preload, off the critical path (its first
            # real copy otherwise pays the ~1.4us table load)
            cp_eng.append(("scalar",
                           nc.scalar.copy(scratch_sb, warm_sb)))

            ps1 = [psum.tile([PT, DIM], f32, tag=f"ps1_{kt}",
                             name=f"ps1_{kt}") for kt in range(NT)]
            last_mt = -1
            W_A = 3 * PT  # 372: regions 0-2 of the last band
            ps2 = [psum.tile([PT, DIM], f32, tag=f"ps2_{mt}",
                             name=f"ps2_{mt}") for mt in range(NT)]

            def mm1(kt, mt):
                mm_is.append(nc.tensor.matmul(
                    ps1[kt][:, mt * PT:(mt + 1) * PT],
                    inp_sb[:, kt, mt * PT:(mt + 1) * PT],
                    inp_sb[:, kt, DIM:ROW],
                    start=True, stop=True))

            def _copy(eng, dst, src):
                if eng == "scalar":
                    i = nc.scalar.copy(dst, src)
                else:
                    i = getattr(nc, eng).tensor_copy(dst, src)
                cp_eng.append((eng, i))
                return i

            def at_copy_sweep(kt):
                # regions assigned to one engine are contiguous slots, so a
                # single copy instruction covers them
                mode = CFG["at"][kt]
                engs = MODE_ENGS[mode]
                r = 0
                while r < NT:
                    eng = engs[r]
                    r2 = r
                    while r2 < NT and engs[r2] == eng:
                        r2 += 1
                    e0, s0 = at_slot[(kt, r)]
                    dst = at_tiles[eng][:, s0 * PT:(s0 + (r2 - r)) * PT]
                    i = _copy(eng, dst, ps1[kt][:, r * PT:r2 * PT])
                    eng_cp_order.setdefault(eng, []).append(i)
                    r = r2

            def mm2(mt, kt):
                if mt == last_mt and kt == NT - 1:
                    # final region lands in ps1[0], long since drained, so
                    # the regions-0-2 copy can overlap this matmul
                    dst = ps1[0][:, :PT]
                else:
                    dst = ps2[mt][:, kt * PT:(kt + 1) * PT]
                mm_is.append(nc.tensor.matmul(
                    dst,
                    at_region(mt, kt),
                    inp_sb[:, kt, DIM:ROW],
                    start=True, stop=True))

            KIND_ENG = {"dve16": "vector", "act16": "scalar",
                        "dve64": "vector"}
            eng_cp_order = {}  # engine -> list of copy instrs, program order

            def out_band_copy(mt):
                kind = CFG["out_cp"][mt]
                eng = KIND_ENG[kind]
                if kind.endswith("64"):
                    i = _copy(eng, out_bands[mt][:, :].bitcast(i64),
                              ps2[mt][:, :].bitcast(i64))
                else:
                    i = _copy(eng, out_bands[mt][:, :], ps2[mt][:, :])
                for prev in eng_cp_order.get(eng, []):
                    tile.add_dep_helper(i.ins, prev.ins, False,
                                        "engine copy order")
                eng_cp_order.setdefault(eng, []).append(i)

            for kt in range(NT):
                for mt in range(NT):
                    mm1(kt, mt)
                at_copy_sweep(kt)

            for mt in CFG["g_order"]:
                if mt == last_mt:
                    for kt in range(NT - 1):
                        mm2(mt, kt)
                    # regions 0-2: copy + DMA overlap the final region's mm
                    ia = _copy("gpsimd", ob_last_a[:, :],
                               ps2[mt][:, :W_A])
                    for prev in eng_cp_order.get("gpsimd", []):
                        tile.add_dep_helper(ia.ins, prev.ins, False,
                                            "engine copy order")
                    eng_cp_order.setdefault("gpsimd", []).append(ia)
                    dma_is.append(nc.sync.dma_start(
                        out16_d[mt * PT:(mt + 1) * PT, :W_A],
                        ob_last_a[:, :]))
                    mm2(mt, NT - 1)
                    ib = _copy("vector", ob_last_b[:, :], ps1[0][:, :PT])
                    for prev in eng_cp_order.get("vector", []):
                        tile.add_dep_helper(ib.ins, prev.ins, False,
                                            "engine copy order")
                    eng_cp_order.setdefault("vector", []).append(ib)
                    dma_is.append(nc.scalar.dma_start(
                        out16_d[mt * PT:(mt + 1) * PT, W_A:],
                        ob_last_b[:, :]))
                    continue
                for kt in range(NT):
                    mm2(mt, kt)
                out_band_copy(mt)
                dst = (out32_d if CFG["out_cp"][mt].endswith("64")
                       else out16_d)
                dma_is.append(getattr(nc, CFG["out_dma"][mt]).dma_start(
                    dst[mt * PT:(mt + 1) * PT, :], out_bands[mt][:, :]))

            # --- pre-drain sem observation (single-wait NOPs on SP) ---
            for d in dma_is:
                n = nc.sync.nop(nofuse=True)
                tile.add_dep_helper(n.ins, d.ins, True, "pre-drain observe")
            groups = {}
            for eng, i in cp_eng:
                groups.setdefault(eng, []).append(i)
            for group in list(groups.values()) + [mm_is]:
                n = nc.sync.nop(nofuse=True)
                for d in group:
                    tile.add_dep_helper(n.ins, d.ins, True, "pre-drain observe")

    return nc


def _in_maps(input_state: np.ndarray, angles: np.ndarray) -> list[dict]:
    u = _build_u(np.asarray(angles, np.float64))
    bt = u[_PERM][:, _PERM].T.astype(np.float16)
    rho = np.asarray(input_state, np.float32)[:, _PERM][:, :, _PERM]
    rho = rho.astype(np.float16)
    out = []
    for b in range(N_CORES):
        inp = np.empty((DIM, ROW), np.float16)
        inp[:, :DIM] = rho[b]
        for kt in range(NT):
            band = slice(kt * PT, (kt + 1) * PT)
            inp[band, DIM:] = bt[band, band]
        out.append({"inp": inp})
    return out


def _unpack_out(raw32: np.ndarray, raw16: np.ndarray) -> np.ndarray:
    o = np.empty((DIM, DIM), np.float32)
    last_mt = CFG["g_order"][-1]
    for mt in range(NT):
        band = slice(mt * PT, (mt + 1) * PT)
        use32 = CFG["out_cp"][mt].endswith("64") and mt != last_mt
        src = raw32 if use32 else raw16
        o[band] = np.asarray(src, np.float32 if use32
                             else np.float16)[band].astype(np.float32)
    return o[_INV_PERM][:, _INV_PERM]


def kernel(input_state: np.ndarray, angles: np.ndarray) -> np.ndarray:
    from concourse.bass_utils import run_bass_kernel_spmd

    if "nc" not in _NC_CACHE:
        _NC_CACHE["nc"] = _build_bass()
    nc = _NC_CACHE["nc"]

    in_maps = _in_maps(input_state, angles)
    res = run_bass_kernel_spmd(nc, in_maps, core_ids=list(range(N_CORES)))
    out = np.stack([_unpack_out(res.results[b].get("out32"),
                                res.results[b]["out16"])
                    for b in range(N_CORES)], axis=0)
    return np.ascontiguousarray(out).astype(np.float32)


# revision 6
# speedup vs baseline: 1.0207x; 1.0207x over previous
"""Trainium2 kernel for nn_Dense_RBS_density: rho <- U rho U^T over a batch
of 8 density matrices in the Hamming-weight-2 basis of 32 qubits (dim=496).

The 15 RBS gates act on disjoint qubit pairs, so they fold into one
orthogonal matrix U (host-built). In a permuted basis U is block-diagonal
with <=4x4 blocks, so each 124-row band only needs its own 124x124
diagonal block of U: per core (one batch element), two passes of 16
independent 124-free fp16 matmuls instead of dense 496^3 work.

Per core (one batch element), with rho' = P rho P^T and block-diagonal
B = P U P^T (host-built, fp16):
  mm1: A^T regions = rho'[kt, mt]^T @ B^T[kt]   (16 independent fp16 matmuls)
  at-copies: ps1 regions -> at_sb fp16, region granular on DVE/ACT/Pool
  mm2: out regions = at[mt, kt]^T @ B^T[kt]     (16 independent fp16 matmuls)
  out-copies: ps2 regions -> out_sb fp16, region granular
  out DMAs: one per band, fp16, split over SP/ACT
"""

import itertools
import math

import numpy as np

N_QUBITS = 32
LIST_GATES = [(2 * i, 2 * i + 1) for i in range(15)]
DIM = 496
PT = 124
NT = 4
N_CORES = 8
ROW = DIM + PT  # 620 packed input row: 496 rho' cols + 124 B^T block cols


def _gate_pairs():
    pairs = list(itertools.combinations(range(N_QUBITS), 2))
    idx = {p: k for k, p in enumerate(pairs)}
    out = []
    for a, b in LIST_GATES:
        rot = []
        for p, k in idx.items():
            if (a in p) and (b not in p):
                other = p[0] if p[1] == a else p[1]
                kp = idx[tuple(sorted((other, b)))]
                rot.append((k, kp))
        out.append(rot)
    return out


_GATE_PAIRS = _gate_pairs()


def _build_perm():
    pairs = list(itertools.combinations(range(N_QUBITS), 2))
    idx = {p: k for k, p in enumerate(pairs)}
    perm = []
    for a in range(15):
        for b in range(a + 1, 15):
            for x in range(2):
                for y in range(2):
                    perm.append(idx[(2 * a + x, 2 * b + y)])
    for q in (30, 31):
        for a in range(15):
            perm.append(idx[tuple(sorted((2 * a, q)))])
            perm.append(idx[tuple(sorted((2 * a + 1, q)))])
    for a in range(15):
        perm.append(idx[(2 * a, 2 * a + 1)])
    perm.append(idx[(30, 31)])
    return np.array(perm)


_PERM = _build_perm()
_INV_PERM = np.argsort(_PERM)


def _build_u(angles: np.ndarray) -> np.ndarray:
    u = np.eye(DIM, dtype=np.float64)
    for g, rot in enumerate(_GATE_PAIRS):
        c = math.cos(float(angles[g]))
        s = math.sin(float(angles[g]))
        k = np.array([r[0] for r in rot])
        kp = np.array([r[1] for r in rot])
        rk, rkp = u[k].copy(), u[kp].copy()
        u[k] = c * rk + s * rkp
        u[kp] = -s * rk + c * rkp
    return u


_NC_CACHE = {}

# schedule knobs (tuned by search.py)
CFG = {
    # at-copy mode per sweep: "half" (DVE regions 0-1 + ACT regions 2-3),
    # "dve" or "act" (whole sweep on one engine). GPSIMD cannot read PSUM
    # on real TRN2, so only DVE/ACT move data out of PSUM.
    "at": ("dve", "act", "dve", "act"),
    # out band-copy engine: dve16 or act16 (single instruction per band)
    "out_cp": ("act16", "dve16", "act16", "dve16"),
    # out band -> DMA engine (sync/scalar/gpsimd)
    "out_dma": ("sync", "gpsimd", "sync", "scalar"),
    # input chunk -> DMA engine, in emission order
    "in_dma": ("sync", "scalar", "gpsimd", "sync"),
    # PE emission order of mm2 groups
    "g_order": (0, 2, 1, 3),
}


def _build_bass():
    import concourse.bass as bass
    import concourse.mybir as mybir
    import concourse.tile as tile
    from concourse.bass import MemorySpace

    f16 = mybir.dt.float16
    f32 = mybir.dt.float32

    nc = bass.Bass("TRN2", target_bir_lowering=False, debug=False)
    inp_d = nc.dram_tensor("inp", [DIM, ROW], f16, kind="ExternalInput").ap()
    use32 = any(k.endswith("64") for k in CFG["out_cp"])
    out32_d = (nc.dram_tensor("out32", [DIM, DIM], f32,
                              kind="ExternalOutput").ap() if use32 else None)
    out16_d = nc.dram_tensor("out16", [DIM, DIM], f16,
                             kind="ExternalOutput").ap()

    with tile.TileContext(nc) as tc:
        with (
            tc.tile_pool(name="consts", bufs=1) as consts,
            tc.tile_pool(name="psum", bufs=1, space=MemorySpace.PSUM) as psum,
        ):
            i64 = mybir.dt.int64
            inp_sb = consts.tile([PT, NT, ROW], f16, tag="inp")
            # slot map: (kt, r) -> (engine, slot index within engine tile)
            MODE_ENGS = {"half": ("vector", "vector", "scalar", "scalar"),
                         "dve": ("vector",) * 4, "act": ("scalar",) * 4}
            at_slot = {}
            eng_nslots = {}
            for kt in range(NT):
                for r in range(NT):
                    eng = MODE_ENGS[CFG["at"][kt]][r]
                    at_slot[(kt, r)] = (eng, eng_nslots.get(eng, 0))
                    eng_nslots[eng] = eng_nslots.get(eng, 0) + 1
            at_tiles = {
                eng: consts.tile([PT, n * PT], f16,
                                 tag=f"at_{eng}", name=f"at_{eng}")
                for eng, n in eng_nslots.items()
            }
            out_bands = [
                consts.tile([PT, DIM],
                            f32 if CFG["out_cp"][mt].endswith("64") else f16,
                            tag=f"ob{mt}", name=f"ob{mt}")
                for mt in range(NT)
            ]
            ob_last_a = consts.tile([PT, 3 * PT], f16, tag="ob_la",
                                    name="ob_la")
            ob_last_b = consts.tile([PT, PT], f16, tag="ob_lb",
                                    name="ob_lb")
            warm_sb = consts.tile([PT, 8], f32, tag="warm")
            scratch_sb = consts.tile([PT, 8], f32, tag="scratch")

            def at_region(kt, r):
                eng, slot = at_slot[(kt, r)]
                return at_tiles[eng][:, slot * PT:(slot + 1) * PT]

            dma_is, mm_is = [], []
            cp_eng = []  # (engine_name, instruction) for drain NOPs

            # --- head ---
            warm_i = nc.vector.memset(warm_sb, 0.0)
            cp_eng.append(("vector", warm_i))

            for kt in range(NT):
                eng = CFG["in_dma"][kt]
                dma_is.append(getattr(nc, eng).dma_start(
                    inp_sb[:, kt, :], inp_d[PT * kt:PT * (kt + 1), :]))

            # ACT activation-table preload, off the critical path (its first
            # real copy otherwise pays the ~1.4us table load)
            cp_eng.append(("scalar",
                           nc.scalar.copy(scratch_sb, warm_sb)))

            ps1 = [psum.tile([PT, DIM], f32, tag=f"ps1_{kt}",
                             name=f"ps1_{kt}") for kt in range(NT)]
            last_mt = -1
            W_A = 3 * PT  # 372: regions 0-2 of the last band
            ps2 = [psum.tile([PT, DIM], f32, tag=f"ps2_{mt}",
                             name=f"ps2_{mt}") for mt in range(NT)]

            def mm1(kt, mt):
                mm_is.append(nc.tensor.matmul(
                    ps1[kt][:, mt * PT:(mt + 1) * PT],
                    inp_sb[:, kt, mt * PT:(mt + 1) * PT],
                    inp_sb[:, kt, DIM:ROW],
                    start=True, stop=True))

            def _copy(eng, dst, src):
                if eng == "scalar":
                    i = nc.scalar.copy(dst, src)
                else:
                    i = getattr(nc, eng).tensor_copy(dst, src)
                cp_eng.append((eng, i))
                return i

            def at_copy_sweep(kt):
                # regions assigned to one engine are contiguous slots, so a
                # single copy instruction covers them
                mode = CFG["at"][kt]
                engs = MODE_ENGS[mode]
                r = 0
                while r < NT:
                    eng = engs[r]
                    r2 = r
                    while r2 < NT and engs[r2] == eng:
                        r2 += 1
                    e0, s0 = at_slot[(kt, r)]
                    dst = at_tiles[eng][:, s0 * PT:(s0 + (r2 - r)) * PT]
                    i = _copy(eng, dst, ps1[kt][:, r * PT:r2 * PT])
                    eng_cp_order.setdefault(eng, []).append(i)
                    r = r2

            def mm2(mt, kt):
                if mt == last_mt and kt == NT - 1:
                    # final region lands in ps1[0], long since drained, so
                    # the regions-0-2 copy can overlap this matmul
                    dst = ps1[0][:, :PT]
                else:
                    dst = ps2[mt][:, kt * PT:(kt + 1) * PT]
                mm_is.append(nc.tensor.matmul(
                    dst,
                    at_region(mt, kt),
                    inp_sb[:, kt, DIM:ROW],
                    start=True, stop=True))

            KIND_ENG = {"dve16": "vector", "act16": "scalar",
                        "dve64": "vector"}
            eng_cp_order = {}  # engine -> list of copy instrs, program order

            def out_band_copy(mt):
                kind = CFG["out_cp"][mt]
                eng = KIND_ENG[kind]
                if kind.endswith("64"):
                    i = _copy(eng, out_bands[mt][:, :].bitcast(i64),
                              ps2[mt][:, :].bitcast(i64))
                else:
                    i = _copy(eng, out_bands[mt][:, :], ps2[mt][:, :])
                for prev in eng_cp_order.get(eng, []):
                    tile.add_dep_helper(i.ins, prev.ins, False,
                                        "engine copy order")
                eng_cp_order.setdefault(eng, []).append(i)

            for kt in range(NT):
                for mt in range(NT):
                    mm1(kt, mt)
                at_copy_sweep(kt)

            for mt in CFG["g_order"]:
                if mt == last_mt:
                    for kt in range(NT - 1):
                        mm2(mt, kt)
                    # regions 0-2: copy + DMA overlap the final region's mm
                    ia = _copy("gpsimd", ob_last_a[:, :],
                               ps2[mt][:, :W_A])
                    for prev in eng_cp_order.get("gpsimd", []):
                        tile.add_dep_helper(ia.ins, prev.ins, False,
                                            "engine copy order")
                    eng_cp_order.setdefault("gpsimd", []).append(ia)
                    dma_is.append(nc.sync.dma_start(
                        out16_d[mt * PT:(mt + 1) * PT, :W_A],
                        ob_last_a[:, :]))
                    mm2(mt, NT - 1)
                    ib = _copy("vector", ob_last_b[:, :], ps1[0][:, :PT])
                    for prev in eng_cp_order.get("vector", []):
                        tile.add_dep_helper(ib.ins, prev.ins, False,
                                            "engine copy order")
                    eng_cp_order.setdefault("vector", []).append(ib)
                    dma_is.append(nc.scalar.dma_start(
                        out16_d[mt * PT:(mt + 1) * PT, W_A:],
                        ob_last_b[:, :]))
                    continue
                for kt in range(NT):
                    mm2(mt, kt)
                out_band_copy(mt)
                dst = (out32_d if CFG["out_cp"][mt].endswith("64")
                       else out16_d)
                dma_is.append(getattr(nc, CFG["out_dma"][mt]).dma_start(
                    dst[mt * PT:(mt + 1) * PT, :], out_bands[mt][:, :]))

            # --- pre-drain sem observation (single-wait NOPs on SP) ---
            for d in dma_is:
                n = nc.sync.nop(nofuse=True)
                tile.add_dep_helper(n.ins, d.ins, True, "pre-drain observe")
            groups = {}
            for eng, i in cp_eng:
                groups.setdefault(eng, []).append(i)
            for group in list(groups.values()) + [mm_is]:
                n = nc.sync.nop(nofuse=True)
                for d in group:
                    tile.add_dep_helper(n.ins, d.ins, True, "pre-drain observe")

    return nc


def _in_maps(input_state: np.ndarray, angles: np.ndarray) -> list[dict]:
    u = _build_u(np.asarray(angles, np.float64))
    bt = u[_PERM][:, _PERM].T.astype(np.float16)
    rho = np.asarray(input_state, np.float32)[:, _PERM][:, :, _PERM]
    rho = rho.astype(np.float16)
    out = []
    for b in range(N_CORES):
        inp = np.empty((DIM, ROW), np.float16)
        inp[:, :DIM] = rho[b]
        for kt in range(NT):
            band = slice(kt * PT, (kt + 1) * PT)
            inp[band, DIM:] = bt[band, band]
        out.append({"inp": inp})
    return out


def _unpack_out(raw32: np.ndarray, raw16: np.ndarray) -> np.ndarray:
    o = np.empty((DIM, DIM), np.float32)
    last_mt = CFG["g_order"][-1]
    for mt in range(NT):
        band = slice(mt * PT, (mt + 1) * PT)
        use32 = CFG["out_cp"][mt].endswith("64") and mt != last_mt
        src = raw32 if use32 else raw16
        o[band] = np.asarray(src, np.float32 if use32
                             else np.float16)[band].astype(np.float32)
    return o[_INV_PERM][:, _INV_PERM]


def kernel(input_state: np.ndarray, angles: np.ndarray) -> np.ndarray:
    from concourse.bass_utils import run_bass_kernel_spmd

    if "nc" not in _NC_CACHE:
        _NC_CACHE["nc"] = _build_bass()
    nc = _NC_CACHE["nc"]

    in_maps = _in_maps(input_state, angles)
    res = run_bass_kernel_spmd(nc, in_maps, core_ids=list(range(N_CORES)))
    out = np.stack([_unpack_out(res.results[b].get("out32"),
                                res.results[b]["out16"])
                    for b in range(N_CORES)], axis=0)
    return np.ascontiguousarray(out).astype(np.float32)
